# revision 15
# baseline (speedup 1.0000x reference)
"""Self-contained Trainium2 Bass kernel for gated attention (sparse_attention).

Reference computation (per batch b):
    q = split_heads(x @ Wq) * DH**-0.5        # (H, n, DH)
    k, v = split_heads(x @ Wkv)               # (H, n, DH) each
    dots = q k^T + attn_bias ; masked softmax over j
    out = (attn @ v) reshaped to (n, H*DH)
    out = out * sigmoid(x @ Wg + bg)
    return out @ Wo + bo

Sharding: 8 cores = 4 batches x 2 query-row halves.  Each core computes
k/v for its full batch (duplicated within the pair) and its own 512
query rows end-to-end, so per-core outputs are disjoint and no
collectives are needed.  The j axis (keys/values) is rolled per-core so
each core's own rows come first, letting one SPMD graph serve all cores.

Device dataflow (per core, all matmuls bf16 into f32 PSUM):
  xT  = dma-transpose(x)                      [c, n]
  kT, v, qT, gatesT via projections           (contract over c)
  per head: dotsT[j,i] = kT^T qT  (K=64, head pairs packed in PE
  row-groups); attnT = exp(dotsT) * ebias (host passes exp(bias));
  AV + row-sum fused in PE column groups; softmax 1/sum is broadcast
  with a rank-1 matmul and folded into the gating multiply.
"""
import sys
import types

import numpy as np
import ml_dtypes

# ---------------------------------------------------------------------------
# Environment shims (axon container): NTFF profile hook + walrus wait fix.
# ---------------------------------------------------------------------------


def _install_axon_ntff_hook():
    try:
        import antenv
    except ImportError:
        return
    if hasattr(antenv, "axon_hooks"):
        return
    mod = types.ModuleType("antenv.axon_hooks")
    mod._hook = None

    def set_axon_ntff_profile_hook(h):
        mod._hook = h

    def get_axon_ntff_profile_hook():
        return mod._hook

    mod.set_axon_ntff_profile_hook = set_axon_ntff_profile_hook
    mod.get_axon_ntff_profile_hook = get_axon_ntff_profile_hook
    sys.modules["antenv.axon_hooks"] = mod
    antenv.axon_hooks = mod
    try:
        from trn_agent_boot.trn_boot import _ntff_profile_via_ctypes

        hook = _ntff_profile_via_ctypes("/opt/axon/libaxon_pjrt.so")
        if hook is not None:
            set_axon_ntff_profile_hook(hook)
    except Exception:
        pass


_install_axon_ntff_hook()

import concourse.bass as bass  # noqa: E402
import concourse.tile as tile  # noqa: E402
import concourse.mybir as mybir  # noqa: E402
from concourse.bass_utils import run_bass_kernel_spmd  # noqa: E402
from concourse.tile import ScopedClock  # noqa: E402


def _patch_tile_drain():
    """The installed walrus accepts only one sync-wait per Drain; Tile's
    tail drain carries one wait per outstanding semaphore.  Split them
    across a chain of single-wait drains (same engine => same semantics)."""

    def _drain_and_barrier(self, tick_clock, wait_clock):
        nc = self.nc
        drain_inst = nc.sync.drain()
        wait_clock.add_sem_waits(
            drain_inst.ins, ScopedClock({None: tick_clock.global_clock})
        )
        si = drain_inst.ins.sync_info
        if si is not None and len(si.on_wait) > 1:
            waits = list(si.on_wait)
            drain_inst.ins.sync_info = mybir.SyncInfo(
                on_wait=waits[:1], on_update=list(si.on_update)
            )
            for w in waits[1:]:
                extra = nc.sync.drain()
                extra.ins.sync_info = mybir.SyncInfo(on_wait=[w], on_update=[])

        nc.all_engine_barrier()
        assert self.sems is not None
        popped = nc._tile_sem_poison_stack.pop()
        assert popped is self._sem_poison
        nc.clear_and_free_semaphores(list(self.sems.allocated().values()))
        nc.all_engine_barrier()

    tile.TileContext._drain_and_barrier = _drain_and_barrier


_patch_tile_drain()


def _legalize_waits(nc, max_waits=1):
    """Walrus in this container accepts at most one sync-wait per lowered
    instruction.  Move surplus waits onto single-wait NoOps inserted just
    before the instruction on the same engine (equivalent semantics: the
    engine blocks on each condition in turn)."""
    nid = 0
    n_split = 0
    for f in nc.m.functions:
        for bb in f.blocks:
            out = []
            changed = False
            for inst in bb.instructions:
                si = inst.sync_info
                if si is not None and len(si.on_wait) > max_waits:
                    waits = list(si.on_wait)
                    for w in waits[:-1]:
                        nop = mybir.InstNoOp(name=f"WSPLIT-{nid}")
                        nid += 1
                        nop.engine = inst.engine
                        nop.sync_info = mybir.SyncInfo(on_wait=[w], on_update=[])
                        out.append(nop)
                    inst.sync_info = mybir.SyncInfo(
                        on_wait=[waits[-1]], on_update=list(si.on_update)
                    )
                    changed = True
                    n_split += 1
                out.append(inst)
            if changed:
                bb.instructions = out
    return n_split


# ---------------------------------------------------------------------------
# Problem constants (hardcoded per spec).
# ---------------------------------------------------------------------------
B, N, D = 4, 1024, 1024
H, DH = 8, 64
INNER = H * DH  # 512
M = N // 2  # 512 query rows per core
N_CORES = 8
P = 128
F32 = mybir.dt.float32
BF16 = mybir.dt.bfloat16


def _build_graph():
    nc = bass.Bass()
    x_ext = nc.declare_dram_parameter("x", [N, D], BF16, isOutput=False)
    bias_ext = nc.declare_dram_parameter("bias", [H, N, M], BF16, isOutput=False)
    wq_ext = nc.declare_dram_parameter("wq", [D, INNER], BF16, isOutput=False)
    wkv_ext = nc.declare_dram_parameter("wkv", [D, 2 * INNER], BF16, isOutput=False)
    wg_ext = nc.declare_dram_parameter("wg", [D, INNER], BF16, isOutput=False)
    bg_ext = nc.declare_dram_parameter("nbg", [P, INNER // P], F32, isOutput=False)
    wo_ext = nc.declare_dram_parameter("wo", [INNER, D], BF16, isOutput=False)
    bo_ext = nc.declare_dram_parameter("bo", [1, D], F32, isOutput=False)
    out_ext = nc.declare_dram_parameter("out", [M, D], F32, isOutput=True)

    CT = D // P  # 8 contraction tiles over feature dim
    DT = INNER // P  # 4 tiles over inner dim (also head pairs)
    NT = N // P  # 8 tiles over sequence
    IB = M // P  # 4 tiles over query rows

    def _copy(out, in_, use_act):
        if use_act:
            nc.scalar.copy(out=out, in_=in_)
        else:
            nc.vector.tensor_copy(out=out, in_=in_)

    with tile.TileContext(nc) as tc:
        with (
            tc.tile_pool(name="persist", bufs=1) as persist,
            tc.tile_pool(name="small", bufs=1) as small,
        ):
            # Long-lived SBUF tensors.
            xT = persist.tile([P, CT, N], BF16)  # x^T: [c, n]
            kT = persist.tile([P, DT, N], BF16)  # k^T: [dI, j]
            v_sb = persist.tile([P, NT, INNER], BF16)  # v:  [j, dI]
            qT = persist.tile([P, DT, M], BF16)  # q^T (scaled): [dI, i]
            gT = persist.tile([P, DT, M], F32)  # gates^T: [dI, i]
            outT = persist.tile([P, DT, M], F32)  # attn-out^T: [dI, i]
            gatedT = persist.tile([P, DT, M], BF16)

            ones_row = small.tile([1, P], F32)
            nc.vector.memset(ones_row, 1.0)
            ones_all = small.tile([P, 64], F32)  # rank-1 lhsT at any base
            nc.vector.memset(ones_all, 1.0)
            ones_col_bf = small.tile([P, 1], BF16)
            nc.vector.memset(ones_col_bf, 1.0)
            bg_sb = small.tile([P, DT], F32)
            nc.sync.dma_start(out=bg_sb, in_=bg_ext[:])
            bo_sb = small.tile([1, D], F32)
            nc.sync.dma_start(out=bo_sb, in_=bo_ext[:])
            sums_sb = small.tile([P, 2, M], F32)  # row sums (p64: h0, p0: h1)

            # ---------------- Phase 1: load, transpose x, project
            with (
                tc.tile_pool(name="wpool", bufs=1) as wpool,
                tc.tile_pool(name="ppool", bufs=4, space="PSUM") as ppool,
            ):
                # x^T straight from DRAM via the DMA xbar transpose.
                for nt in range(NT):
                    nc.sync.dma_start_transpose(
                        xT[:, :, nt * P : (nt + 1) * P],
                        x_ext[nt * P : (nt + 1) * P, :],
                    )
                wq_sb = wpool.tile([P, CT, INNER], BF16)
                wkv_sb = wpool.tile([P, CT, 2 * INNER], BF16)
                wg_sb = wpool.tile([P, CT, INNER], BF16)
                for ct in range(CT):
                    nc.sync.dma_start(
                        out=wkv_sb[:, ct, :], in_=wkv_ext[ct * P : (ct + 1) * P, :]
                    )
                for ct in range(CT):
                    nc.scalar.dma_start(
                        out=wq_sb[:, ct, :], in_=wq_ext[ct * P : (ct + 1) * P, :]
                    )
                    nc.scalar.dma_start(
                        out=wg_sb[:, ct, :], in_=wg_ext[ct * P : (ct + 1) * P, :]
                    )

                # kT[dI, j]: lhsT = Wk[c, dI-block], rhs = xT[c, j-half]
                for dt in range(DT):
                    for jh in range(2):
                        pk = ppool.tile([P, 512], F32, tag="pk")
                        for ct in range(CT):
                            nc.tensor.matmul(
                                pk,
                                lhsT=wkv_sb[:, ct, dt * P : (dt + 1) * P],
                                rhs=xT[:, ct, jh * 512 : (jh + 1) * 512],
                                start=(ct == 0),
                                stop=(ct == CT - 1),
                            )
                        _copy(kT[:, dt, jh * 512 : (jh + 1) * 512], pk, True)

                # v[j, dI] (bf16): lhsT = xT[c, j-block], rhs = Wv[c, :]
                for jt in range(NT):
                    pv = ppool.tile([P, INNER], F32, tag="pk")
                    for ct in range(CT):
                        nc.tensor.matmul(
                            pv,
                            lhsT=xT[:, ct, jt * P : (jt + 1) * P],
                            rhs=wkv_sb[:, ct, INNER:],
                            start=(ct == 0),
                            stop=(ct == CT - 1),
                        )
                    _copy(v_sb[:, jt, :], pv, True)

                # qT[dI, i] (scale folded into Wq on host)
                for dt in range(DT):
                    pq = ppool.tile([P, M], F32, tag="pk")
                    for ct in range(CT):
                        nc.tensor.matmul(
                            pq,
                            lhsT=wq_sb[:, ct, dt * P : (dt + 1) * P],
                            rhs=xT[:, ct, :M],
                            start=(ct == 0),
                            stop=(ct == CT - 1),
                        )
                    _copy(qT[:, dt, :], pq, True)

                # gatesT[dI, i] = sigmoid(x @ Wg + bg)^T
                for dt in range(DT):
                    pg = ppool.tile([P, M], F32, tag="pk")
                    for ct in range(CT):
                        nc.tensor.matmul(
                            pg,
                            lhsT=wg_sb[:, ct, dt * P : (dt + 1) * P],
                            rhs=xT[:, ct, :M],
                            start=(ct == 0),
                            stop=(ct == CT - 1),
                        )
                    nc.scalar.activation(
                        out=gT[:, dt, :],
                        in_=pg,
                        func=mybir.ActivationFunctionType.Sigmoid,
                        scale=1.0,
                        bias=bg_sb[:, dt : dt + 1],
                    )

            # ---------------- Phases 2+3 share the fpool scope so the Wo
            # weights stream in while attention runs.
            with tc.tile_pool(name="fpool", bufs=1) as fpool:
                wo_sb = fpool.tile([P, DT, D], BF16)
                for dt in range(DT):
                    nc.scalar.dma_start(
                        out=wo_sb[:, dt, :], in_=wo_ext[dt * P : (dt + 1) * P, :]
                    )
                out_sb = fpool.tile([P, IB, D], F32)

                # ---------------- Phase 2: attention, head pairs
                with (
                    tc.tile_pool(name="apool", bufs=2) as apool,
                    tc.tile_pool(name="pdots", bufs=2, space="PSUM") as pdots,
                    tc.tile_pool(name="psums", bufs=1, space="PSUM") as psums,
                    tc.tile_pool(name="pav", bufs=1, space="PSUM") as pav,
                ):
                    for dt in range(DT):  # head pair (2*dt, 2*dt+1)
                        biasT_sb = apool.tile([P, 2, NT, M], BF16, tag="bias")
                        nc.sync.dma_start(
                            out=biasT_sb,
                            in_=bias_ext[2 * dt : 2 * dt + 2].rearrange(
                                "h (jt p) i -> p h jt i", p=P
                            ),
                        )
                        attnT = {}
                        for hi in range(2):
                            a_tile = apool.tile(
                                [P, NT, M], BF16, tag=f"attnT{hi}", name=f"aT{hi}"
                            )
                            attnT[hi] = a_tile
                        # Interleaved dots: the two heads occupy disjoint
                        # PE row-groups (K=64 at partition 0 / 64), so
                        # adjacent issues run concurrently on the array.
                        for jt in range(NT):
                            for hi in range(2):
                                po = 64 * hi
                                pd = pdots.tile([P, M], F32, tag=f"pd{hi}")
                                nc.tensor.matmul(
                                    pd,
                                    lhsT=kT[po : po + 64, dt, jt * P : (jt + 1) * P],
                                    rhs=qT[po : po + 64, dt, :],
                                    start=True,
                                    stop=True,
                                )
                                nc.scalar.activation(
                                    out=attnT[hi][:, jt, :],
                                    in_=pd,
                                    func=mybir.ActivationFunctionType.Exp,
                                )
                                # bias folded multiplicatively: host passes
                                # exp(bias), so attn = exp(qk) * exp(bias)
                                nc.vector.tensor_tensor(
                                    attnT[hi][:, jt, :],
                                    attnT[hi][:, jt, :],
                                    biasT_sb[:, hi, jt, :],
                                    mybir.AluOpType.mult,
                                )

                        # AV + row-sums fused: per head one 64-wide AV in PE
                        # column groups plus a ones-column matmul in a spare
                        # column group of the same PSUM tile.
                        # t0: rows 0-63 = out h0, row 64 = sums h0
                        # t1: rows 64-127 = out h1, row 0 = sums h1
                        t0 = pav.tile([P, M], F32, tag="av0")
                        t1 = pav.tile([P, M], F32, tag="av1")
                        h0, h1 = 2 * dt, 2 * dt + 1
                        for jt in range(NT):
                            st = jt == 0
                            sp = jt == NT - 1
                            nc.tensor.matmul(
                                t0[0:64, :],
                                lhsT=v_sb[:, jt, h0 * 64 : h0 * 64 + 64],
                                rhs=attnT[0][:, jt, :],
                                start=st,
                                stop=sp,
                                tile_position=(0, 0),
                                skip_group_check=True,
                            )
                            nc.tensor.matmul(
                                t0[64:65, :],
                                lhsT=ones_col_bf,
                                rhs=attnT[0][:, jt, :],
                                start=st,
                                stop=sp,
                                tile_position=(0, 64),
                                skip_group_check=True,
                            )
                            nc.tensor.matmul(
                                t1[64:128, :],
                                lhsT=v_sb[:, jt, h1 * 64 : h1 * 64 + 64],
                                rhs=attnT[1][:, jt, :],
                                start=st,
                                stop=sp,
                                tile_position=(0, 64),
                                skip_group_check=True,
                            )
                            nc.tensor.matmul(
                                t1[0:1, :],
                                lhsT=ones_col_bf,
                                rhs=attnT[1][:, jt, :],
                                start=st,
                                stop=sp,
                                tile_position=(0, 0),
                                skip_group_check=True,
                            )
                        _copy(outT[0:64, dt, :], t0[0:64, :], False)
                        _copy(outT[64:128, dt, :], t1[64:128, :], False)
                        nc.scalar.copy(out=sums_sb[64:65, 0, :], in_=t0[64:65, :])
                        nc.scalar.copy(out=sums_sb[0:1, 1, :], in_=t1[0:1, :])

                        # Broadcast sums across the pair's partitions with
                        # rank-1 matmuls, then 1/x and fold into gating.
                        prf = psums.tile([P, M], F32, tag="prf")
                        nc.tensor.matmul(
                            prf[0:64, :],
                            lhsT=ones_all[64:65, :],
                            rhs=sums_sb[64:65, 0, :],
                            start=True,
                            stop=True,
                            tile_position=(64, 0),
                            skip_group_check=True,
                        )
                        nc.tensor.matmul(
                            prf[64:128, :],
                            lhsT=ones_all[0:1, :],
                            rhs=sums_sb[0:1, 1, :],
                            start=True,
                            stop=True,
                            tile_position=(0, 64),
                            skip_group_check=True,
                        )
                        nc.vector.reciprocal(out=prf, in_=prf)
                        nc.vector.tensor_tensor(
                            outT[:, dt, :],
                            outT[:, dt, :],
                            prf,
                            mybir.AluOpType.mult,
                        )
                        nc.vector.tensor_tensor(
                            gatedT[:, dt, :],
                            outT[:, dt, :],
                            gT[:, dt, :],
                            mybir.AluOpType.mult,
                        )

                # ---------------- Phase 3: output projection (+bo)
                with tc.tile_pool(name="pf", bufs=4, space="PSUM") as pf:
                    for ib in range(IB):
                        for dh in range(2):
                            po_t = pf.tile([P, 512], F32, tag="pf")
                            nc.tensor.matmul(
                                po_t,
                                lhsT=ones_row,
                                rhs=bo_sb[:, dh * 512 : (dh + 1) * 512],
                                start=True,
                                stop=False,
                                skip_group_check=True,
                            )
                            for dt in range(DT):
                                nc.tensor.matmul(
                                    po_t,
                                    lhsT=gatedT[:, dt, ib * P : (ib + 1) * P],
                                    rhs=wo_sb[:, dt, dh * 512 : (dh + 1) * 512],
                                    start=False,
                                    stop=(dt == DT - 1),
                                    skip_group_check=True,
                                )
                            _copy(
                                out_sb[:, ib, dh * 512 : (dh + 1) * 512],
                                po_t,
                                (ib + dh) % 2 == 0,
                            )
                nc.sync.dma_start(
                    out=out_ext.rearrange("(ib p) d -> p ib d", p=P), in_=out_sb
                )

    _legalize_waits(nc)
    return nc


_NC_CACHE = None


def _get_graph():
    global _NC_CACHE
    if _NC_CACHE is None:
        _NC_CACHE = _build_graph()
    return _NC_CACHE


def _prepare_in_maps(x, mask, attn_bias, Wq, Wkv, Wg, bg, Wo, bo):
    x = np.asarray(x, dtype=np.float32)
    mask = np.asarray(mask, dtype=bool)
    attn_bias = np.asarray(attn_bias, dtype=np.float32)
    Wq = np.asarray(Wq, dtype=np.float32)
    Wkv = np.asarray(Wkv, dtype=np.float32)
    Wg = np.asarray(Wg, dtype=np.float32)
    bg = np.asarray(bg, dtype=np.float32)
    Wo = np.asarray(Wo, dtype=np.float32)
    bo = np.asarray(bo, dtype=np.float32)

    wq_scaled = np.ascontiguousarray(Wq * np.float32(DH**-0.5)).astype(
        ml_dtypes.bfloat16
    )
    bg2 = np.ascontiguousarray(bg.reshape(INNER // P, P).T)
    bo2 = np.ascontiguousarray(bo.reshape(1, D))
    wkv_b = Wkv.astype(ml_dtypes.bfloat16)
    wg_b = Wg.astype(ml_dtypes.bfloat16)
    wo_b = Wo.astype(ml_dtypes.bfloat16)

    # Fold the attention mask into the bias (j side), then exponentiate:
    # the kernel computes attn = exp(qk) * exp(bias).  Masked entries
    # become exactly 0.
    m2 = mask[:, None, :, None] & mask[:, None, None, :]  # (B, 1, n, n)
    bias_eff = np.where(m2, attn_bias, np.float32(-np.inf))
    bias_eff = np.exp(bias_eff)

    in_maps = []
    for c in range(N_CORES):
        b, r = divmod(c, 2)
        x_perm = np.roll(x[b], -r * M, axis=0)
        bias_c = bias_eff[b][:, r * M : (r + 1) * M, :]
        bias_c = np.roll(bias_c, -r * M, axis=2)
        bias_cT = bias_c.transpose(0, 2, 1)  # (H, N, M): j on rows
        in_maps.append(
            {
                "x": np.ascontiguousarray(x_perm).astype(ml_dtypes.bfloat16),
                "bias": np.ascontiguousarray(bias_cT).astype(ml_dtypes.bfloat16),
                "wq": wq_scaled,
                "wkv": wkv_b,
                "wg": wg_b,
                "nbg": bg2,
                "wo": wo_b,
                "bo": bo2,
            }
        )
    return in_maps


def _assemble(results):
    out = np.empty((B, N, D), dtype=np.float32)
    for c in range(N_CORES):
        b, r = divmod(c, 2)
        out[b, r * M : (r + 1) * M, :] = results[c]["out"]
    return out


def kernel(**inputs):
    nc = _get_graph()
    in_maps = _prepare_in_maps(**inputs)
    res = run_bass_kernel_spmd(nc, in_maps, core_ids=list(range(N_CORES)))
    return _assemble(res.results)


def kernel_traced(**inputs):
    """Like kernel() but with NTFF profiling; returns (out, exec_time_ns)."""
    nc = _get_graph()
    in_maps = _prepare_in_maps(**inputs)
    res = run_bass_kernel_spmd(
        nc, in_maps, core_ids=list(range(N_CORES)), trace=True
    )
    return _assemble(res.results), res.exec_time_ns


# revision 21
# speedup vs baseline: 1.5362x; 1.5362x over previous
"""Self-contained Trainium2 Bass kernel for gated attention (sparse_attention).

Reference computation (per batch b):
    q = split_heads(x @ Wq) * DH**-0.5        # (H, n, DH)
    k, v = split_heads(x @ Wkv)               # (H, n, DH) each
    dots = q k^T + attn_bias ; masked softmax over j
    out = (attn @ v) reshaped to (n, H*DH)
    out = out * sigmoid(x @ Wg + bg)
    return out @ Wo + bo

Sharding: 8 cores = 4 batches x 2 query-row halves.  Each core computes
k/v for its full batch (duplicated within the pair) and its own 512
query rows end-to-end, so per-core outputs are disjoint and no
collectives are needed.  The j axis (keys/values) is rolled per-core so
each core's own rows come first, letting one SPMD graph serve all cores.
"""
import sys
import types

import numpy as np
import ml_dtypes

# ---------------------------------------------------------------------------
# Environment shims (axon container): NTFF profile hook + walrus drain fix.
# ---------------------------------------------------------------------------


def _install_axon_ntff_hook():
    try:
        import antenv
    except ImportError:
        return
    if hasattr(antenv, "axon_hooks"):
        return
    mod = types.ModuleType("antenv.axon_hooks")
    mod._hook = None

    def set_axon_ntff_profile_hook(h):
        mod._hook = h

    def get_axon_ntff_profile_hook():
        return mod._hook

    mod.set_axon_ntff_profile_hook = set_axon_ntff_profile_hook
    mod.get_axon_ntff_profile_hook = get_axon_ntff_profile_hook
    sys.modules["antenv.axon_hooks"] = mod
    antenv.axon_hooks = mod
    try:
        from trn_agent_boot.trn_boot import _ntff_profile_via_ctypes

        hook = _ntff_profile_via_ctypes("/opt/axon/libaxon_pjrt.so")
        if hook is not None:
            set_axon_ntff_profile_hook(hook)
    except Exception:
        pass


_install_axon_ntff_hook()

import concourse.bass as bass  # noqa: E402
import concourse.tile as tile  # noqa: E402
import concourse.mybir as mybir  # noqa: E402
from concourse.bass_utils import run_bass_kernel_spmd  # noqa: E402
from concourse.masks import make_identity  # noqa: E402
from concourse.tile import ScopedClock  # noqa: E402


def _patch_tile_drain():
    """The installed walrus accepts only one sync-wait per Drain; Tile's
    tail drain carries one wait per outstanding semaphore.  Split them
    across a chain of single-wait drains (same engine => same semantics)."""

    def _drain_and_barrier(self, tick_clock, wait_clock):
        nc = self.nc
        drain_inst = nc.sync.drain()
        wait_clock.add_sem_waits(
            drain_inst.ins, ScopedClock({None: tick_clock.global_clock})
        )
        si = drain_inst.ins.sync_info
        if si is not None and len(si.on_wait) > 1:
            waits = list(si.on_wait)
            drain_inst.ins.sync_info = mybir.SyncInfo(
                on_wait=waits[:1], on_update=list(si.on_update)
            )
            for w in waits[1:]:
                extra = nc.sync.drain()
                extra.ins.sync_info = mybir.SyncInfo(on_wait=[w], on_update=[])

        nc.all_engine_barrier()
        assert self.sems is not None
        popped = nc._tile_sem_poison_stack.pop()
        assert popped is self._sem_poison
        nc.clear_and_free_semaphores(list(self.sems.allocated().values()))
        nc.all_engine_barrier()

    tile.TileContext._drain_and_barrier = _drain_and_barrier


_patch_tile_drain()


def _legalize_waits(nc, max_waits=1):
    """Walrus in this container accepts at most one sync-wait per lowered
    instruction.  Move surplus waits onto single-wait NoOps inserted just
    before the instruction on the same engine (equivalent semantics: the
    engine blocks on each condition in turn)."""
    nid = 0
    n_split = 0
    for f in nc.m.functions:
        for bb in f.blocks:
            out = []
            changed = False
            for inst in bb.instructions:
                si = inst.sync_info
                if si is not None and len(si.on_wait) > max_waits:
                    waits = list(si.on_wait)
                    for w in waits[:-1]:
                        nop = mybir.InstNoOp(name=f"WSPLIT-{nid}")
                        nid += 1
                        nop.engine = inst.engine
                        nop.sync_info = mybir.SyncInfo(on_wait=[w], on_update=[])
                        out.append(nop)
                    inst.sync_info = mybir.SyncInfo(
                        on_wait=[waits[-1]], on_update=list(si.on_update)
                    )
                    changed = True
                    n_split += 1
                out.append(inst)
            if changed:
                bb.instructions = out
    return n_split


# ---------------------------------------------------------------------------
# Problem constants (hardcoded per spec).
# ---------------------------------------------------------------------------
B, N, D = 4, 1024, 1024
H, DH = 8, 64
INNER = H * DH  # 512
M = N // 2  # 512 query rows per core
N_CORES = 8
P = 128
F32 = mybir.dt.float32
BF16 = mybir.dt.bfloat16


def _build_graph():
    nc = bass.Bass()
    x_ext = nc.declare_dram_parameter("x", [N, D], BF16, isOutput=False)
    bias_ext = nc.declare_dram_parameter("bias", [H // 2, N, 2, M], BF16, isOutput=False)
    wq_ext = nc.declare_dram_parameter("wq", [D, INNER], BF16, isOutput=False)
    wkv_ext = nc.declare_dram_parameter("wkv", [D, 2 * INNER], BF16, isOutput=False)
    wg_ext = nc.declare_dram_parameter("wg", [D, INNER], BF16, isOutput=False)
    nbg_ext = nc.declare_dram_parameter("nbg", [P, INNER // P], F32, isOutput=False)
    wo_ext = nc.declare_dram_parameter("wo", [INNER, D], BF16, isOutput=False)
    bo_ext = nc.declare_dram_parameter("bo", [1, D], F32, isOutput=False)
    out_ext = nc.declare_dram_parameter("out", [M, D], F32, isOutput=True)

    CT = D // P  # 8 contraction tiles over feature dim
    DT = INNER // P  # 4 tiles over inner dim
    NT = N // P  # 8 tiles over sequence
    IB = M // P  # 4 tiles over query rows


    def _copy(out, in_, use_act):
        if use_act:
            nc.scalar.copy(out=out, in_=in_)
        else:
            nc.vector.tensor_copy(out=out, in_=in_)

    with tile.TileContext(nc) as tc:
        with (
            tc.tile_pool(name="persist", bufs=1) as persist,
            tc.tile_pool(name="small", bufs=1) as small,
        ):
            # Long-lived SBUF tensors.
            xT = persist.tile([P, CT, N], BF16)  # x^T: [c, n]
            kT = persist.tile([P, DT, N], BF16)  # k^T: [dI, j]
            v_sb = persist.tile([P, NT, INNER], BF16)  # v:  [j, dI]
            qT = persist.tile([P, DT, M], BF16)  # q^T (scaled): [dI, i]
            gT = persist.tile([P, DT, M], F32)  # gates^T: [dI, i]
            outT = persist.tile([P, DT, M], F32)  # attn-out^T: [dI, i]
            gatedT = persist.tile([P, DT, M], BF16)

            ident = small.tile([P, P], BF16)
            make_identity(nc, ident)
            ones_row = small.tile([1, P], F32)
            nc.vector.memset(ones_row, 1.0)
            nbg_sb = small.tile([P, DT], F32)
            nc.sync.dma_start(out=nbg_sb, in_=nbg_ext[:])
            bo_sb = small.tile([1, D], F32)
            nc.sync.dma_start(out=bo_sb, in_=bo_ext[:])
            ones_col_bf = small.tile([P, 1], BF16)
            nc.vector.memset(ones_col_bf, 1.0)
            srow = small.tile([1, H, M], F32)  # per-head row sums -> 1/sum

            # ---------------- Phase 0+1: load x/weights, transpose, project
            with (
                tc.tile_pool(name="wpool", bufs=1) as wpool,
                tc.tile_pool(name="ppool", bufs=4, space="PSUM") as ppool,
            ):
                x_sb = wpool.tile([P, NT, D], BF16)
                wq_sb = wpool.tile([P, CT, INNER], BF16)
                wkv_sb = wpool.tile([P, CT, 2 * INNER], BF16)
                wg_sb = wpool.tile([P, CT, INNER], BF16)
                for nt in range(NT):
                    nc.sync.dma_start(
                        out=x_sb[:, nt, :], in_=x_ext[nt * P : (nt + 1) * P, :]
                    )
                for ct in range(CT):
                    nc.sync.dma_start(
                        out=wkv_sb[:, ct, :], in_=wkv_ext[ct * P : (ct + 1) * P, :]
                    )
                for ct in range(CT):
                    nc.scalar.dma_start(
                        out=wq_sb[:, ct, :], in_=wq_ext[ct * P : (ct + 1) * P, :]
                    )
                    nc.scalar.dma_start(
                        out=wg_sb[:, ct, :], in_=wg_ext[ct * P : (ct + 1) * P, :]
                    )

                # x^T via PE transpose of 128x128 blocks.
                for ct in range(CT):
                    for nt in range(NT):
                        pt = ppool.tile([P, P], BF16, tag="pt")
                        nc.tensor.transpose(
                            pt, x_sb[:, nt, ct * P : (ct + 1) * P], ident
                        )
                        _copy(xT[:, ct, nt * P : (nt + 1) * P], pt, False)

                # kT[dI, j]: lhsT = Wk[c, dI-block], rhs = xT[c, j-half]
                for dt in range(DT):
                    for jh in range(2):
                        pk = ppool.tile([P, 512], F32, tag="pk")
                        for ct in range(CT):
                            nc.tensor.matmul(
                                pk,
                                lhsT=wkv_sb[:, ct, dt * P : (dt + 1) * P],
                                rhs=xT[:, ct, jh * 512 : (jh + 1) * 512],
                                start=(ct == 0),
                                stop=(ct == CT - 1),
                            )
                        _copy(kT[:, dt, jh * 512 : (jh + 1) * 512], pk, True)

                # v[j, dI] (bf16): lhsT = xT[c, j-block], rhs = Wv[c, :]
                for jt in range(NT):
                    pv = ppool.tile([P, INNER], F32, tag="pk")
                    for ct in range(CT):
                        nc.tensor.matmul(
                            pv,
                            lhsT=xT[:, ct, jt * P : (jt + 1) * P],
                            rhs=wkv_sb[:, ct, INNER:],
                            start=(ct == 0),
                            stop=(ct == CT - 1),
                        )
                    _copy(v_sb[:, jt, :], pv, True)

                # qT[dI, i] (already scale-folded into Wq on host)
                for dt in range(DT):
                    pq = ppool.tile([P, M], F32, tag="pk")
                    for ct in range(CT):
                        nc.tensor.matmul(
                            pq,
                            lhsT=wq_sb[:, ct, dt * P : (dt + 1) * P],
                            rhs=xT[:, ct, :M],
                            start=(ct == 0),
                            stop=(ct == CT - 1),
                        )
                    _copy(qT[:, dt, :], pq, True)

                # gatesT[dI, i] = sigmoid(zT + bg) = 1 / (1 + exp(-zT - bg))
                for dt in range(DT):
                    pg = ppool.tile([P, M], F32, tag="pk")
                    for ct in range(CT):
                        nc.tensor.matmul(
                            pg,
                            lhsT=wg_sb[:, ct, dt * P : (dt + 1) * P],
                            rhs=xT[:, ct, :M],
                            start=(ct == 0),
                            stop=(ct == CT - 1),
                        )
                    nc.scalar.activation(
                        out=gT[:, dt, :],
                        in_=pg,
                        func=mybir.ActivationFunctionType.Sigmoid,
                        scale=1.0,
                        bias=nbg_sb[:, dt : dt + 1],
                    )

            # ---------------- Phase 2: attention, head pairs
            # dots are computed TRANSPOSED (j on partitions) so the exp
            # output is directly in the layout the AV matmul needs --
            # the bias arrives host-transposed.  Row sums come from
            # ones-vector matmuls; 1/sum is folded into the gating.
            with (
                tc.tile_pool(name="apool", bufs=2) as apool,
                tc.tile_pool(name="pdots", bufs=2, space="PSUM") as pdots,
                tc.tile_pool(name="psums", bufs=1, space="PSUM") as psums,
                tc.tile_pool(name="pav", bufs=2, space="PSUM") as pav,
            ):
                for dt in range(DT):  # head pair (2*dt, 2*dt+1)
                    biasT_sb = apool.tile([P, NT, 2, M], BF16, tag="bias")
                    nc.sync.dma_start(
                        out=biasT_sb,
                        in_=bias_ext[dt].rearrange(
                            "(jt p) h i -> p jt h i", p=P
                        ),
                    )
                    aTp = apool.tile([P, NT, 2, M], BF16, tag="attnT")
                    attnT = {0: aTp, 1: aTp}
                    # Interleave the two heads' QK matmuls: disjoint PE
                    # row-groups (K=64 at partition 0/64) run concurrently;
                    # their outputs share one 2-bank PSUM tile so a single
                    # exp + a single ebias-multiply cover both heads.
                    for jt in range(NT):
                        pd2 = pdots.tile([P, 2, M], F32, tag="pd")
                        for hi in range(2):
                            po = 64 * hi
                            nc.tensor.matmul(
                                pd2[:, hi, :],
                                lhsT=kT[po : po + 64, dt, jt * P : (jt + 1) * P],
                                rhs=qT[po : po + 64, dt, :],
                                start=True,
                                stop=True,
                            )
                        nc.scalar.activation(
                            out=aTp[:, jt, :, :],
                            in_=pd2,
                            func=mybir.ActivationFunctionType.Exp,
                        )
                        nc.vector.tensor_tensor(
                            aTp[:, jt, :, :],
                            aTp[:, jt, :, :],
                            biasT_sb[:, jt, :, :],
                            mybir.AluOpType.mult,
                        )
                    for hi in range(2):
                        h = 2 * dt + hi
                        aT = attnT[hi]
                        # row sums: s[i] = sum_j attn -- ones-vector matmul
                        ps = psums.tile([1, M], F32, tag="ps")
                        for jt in range(NT):
                            nc.tensor.matmul(
                                ps,
                                lhsT=ones_col_bf,
                                rhs=aTp[:, jt, hi, :],
                                start=(jt == 0),
                                stop=(jt == NT - 1),
                            )
                        nc.scalar.copy(out=srow[:, h, :], in_=ps)

                    # AV for the pair: col-tiled into one PSUM tile.
                    pav_t = pav.tile([P, M], F32, tag="pav")
                    for hi in range(2):
                        h = 2 * dt + hi
                        for jt in range(NT):
                            nc.tensor.matmul(
                                pav_t[64 * hi : 64 * hi + 64, :],
                                lhsT=v_sb[:, jt, h * 64 : (h + 1) * 64],
                                rhs=aTp[:, jt, hi, :],
                                start=(jt == 0),
                                stop=(jt == NT - 1),
                                tile_position=(0, 64 * hi),
                                skip_group_check=True,
                            )
                    _copy(outT[:, dt, :], pav_t, False)
                    prf = psums.tile([P, M], F32, tag="prf")
                    for hi in range(2):
                        h = 2 * dt + hi
                        nc.tensor.matmul(
                            prf[64 * hi : 64 * hi + 64, :],
                            lhsT=ones_row[:, :64],
                            rhs=srow[:, h, :],
                            start=True,
                            stop=True,
                            tile_position=(0, 64 * hi),
                            skip_group_check=True,
                        )
                    nc.vector.reciprocal(out=prf, in_=prf)
                    nc.vector.tensor_tensor(
                        outT[:, dt, :],
                        outT[:, dt, :],
                        prf,
                        mybir.AluOpType.mult,
                    )
                    nc.vector.tensor_tensor(
                        gatedT[:, dt, :],
                        outT[:, dt, :],
                        gT[:, dt, :],
                        mybir.AluOpType.mult,
                    )

            # ---------------- Phase 3: gating + output projection
            with (
                tc.tile_pool(name="fpool", bufs=1) as fpool,
                tc.tile_pool(name="pf", bufs=4, space="PSUM") as pf,
            ):
                wo_sb = fpool.tile([P, DT, D], BF16)
                for dt in range(DT):
                    nc.scalar.dma_start(
                        out=wo_sb[:, dt, :], in_=wo_ext[dt * P : (dt + 1) * P, :]
                    )
                bo_bcast = fpool.tile([P, D], F32)
                for dh in range(2):
                    pb = pf.tile([P, 512], F32, tag="pf")
                    nc.tensor.matmul(
                        pb,
                        lhsT=ones_row,
                        rhs=bo_sb[:, dh * 512 : (dh + 1) * 512],
                        start=True,
                        stop=True,
                        skip_group_check=True,
                    )
                    _copy(bo_bcast[:, dh * 512 : (dh + 1) * 512], pb, True)
                out_sb = fpool.tile([P, IB, D], F32)
                for ib in range(IB):
                    for dh in range(2):
                        po_t = pf.tile([P, 512], F32, tag="pf")
                        for dt in range(DT):
                            nc.tensor.matmul(
                                po_t,
                                lhsT=gatedT[:, dt, ib * P : (ib + 1) * P],
                                rhs=wo_sb[:, dt, dh * 512 : (dh + 1) * 512],
                                start=(dt == 0),
                                stop=(dt == DT - 1),
                                skip_group_check=True,
                            )
                        nc.vector.tensor_tensor(
                            out_sb[:, ib, dh * 512 : (dh + 1) * 512],
                            po_t,
                            bo_bcast[:, dh * 512 : (dh + 1) * 512],
                            mybir.AluOpType.add,
                        )
                    nc.sync.dma_start(
                        out=out_ext.rearrange("(ib p) d -> p ib d", p=P)[:, ib, :],
                        in_=out_sb[:, ib, :],
                    )

    _legalize_waits(nc)
    return nc


_NC_CACHE = None


def _get_graph():
    global _NC_CACHE
    if _NC_CACHE is None:
        _NC_CACHE = _build_graph()
    return _NC_CACHE


def _prepare_in_maps(x, mask, attn_bias, Wq, Wkv, Wg, bg, Wo, bo):
    x = np.asarray(x, dtype=np.float32)
    mask = np.asarray(mask, dtype=bool)
    attn_bias = np.asarray(attn_bias, dtype=np.float32)
    Wq = np.asarray(Wq, dtype=np.float32)
    Wkv = np.asarray(Wkv, dtype=np.float32)
    Wg = np.asarray(Wg, dtype=np.float32)
    bg = np.asarray(bg, dtype=np.float32)
    Wo = np.asarray(Wo, dtype=np.float32)
    bo = np.asarray(bo, dtype=np.float32)

    wq_scaled = np.ascontiguousarray(Wq * np.float32(DH**-0.5)).astype(
        ml_dtypes.bfloat16
    )
    nbg = np.ascontiguousarray(bg.reshape(INNER // P, P).T)
    bo2 = np.ascontiguousarray(bo.reshape(1, D))
    wkv_b = Wkv.astype(ml_dtypes.bfloat16)
    wg_b = Wg.astype(ml_dtypes.bfloat16)
    wo_b = Wo.astype(ml_dtypes.bfloat16)

    # Fold the attention mask into the bias (j side), then exponentiate:
    # the kernel computes attn = exp(qk) * exp(bias).  Masked entries
    # become exactly 0.
    m2 = mask[:, None, :, None] & mask[:, None, None, :]  # (B, 1, n, n)
    bias_eff = np.where(m2, attn_bias, np.float32(-np.inf))
    bias_eff = np.exp(bias_eff)

    in_maps = []
    for c in range(N_CORES):
        b, r = divmod(c, 2)
        x_perm = np.roll(x[b], -r * M, axis=0)
        bias_c = bias_eff[b][:, r * M : (r + 1) * M, :]
        bias_c = np.roll(bias_c, -r * M, axis=2)
        # (H//2, N, 2, M): head pairs adjacent per j row for one 3D DMA
        bias_cT = bias_c.reshape(H // 2, 2, M, N).transpose(0, 3, 1, 2)
        in_maps.append(
            {
                "x": np.ascontiguousarray(x_perm).astype(ml_dtypes.bfloat16),
                "bias": np.ascontiguousarray(bias_cT).astype(ml_dtypes.bfloat16),
                "wq": wq_scaled,
                "wkv": wkv_b,
                "wg": wg_b,
                "nbg": nbg,
                "wo": wo_b,
                "bo": bo2,
            }
        )
    return in_maps


def _assemble(results):
    out = np.empty((B, N, D), dtype=np.float32)
    for c in range(N_CORES):
        b, r = divmod(c, 2)
        out[b, r * M : (r + 1) * M, :] = results[c]["out"]
    return out


def _run(in_maps, trace=False):
    nc = _get_graph()
    last_err = None
    for attempt in range(3):
        try:
            return run_bass_kernel_spmd(
                nc, in_maps, core_ids=list(range(N_CORES)), trace=trace
            )
        except Exception as e:  # transient device faults recover on retry
            last_err = e
    raise last_err


def kernel(**inputs):
    in_maps = _prepare_in_maps(**inputs)
    res = _run(in_maps)
    return _assemble(res.results)


def kernel_traced(**inputs):
    """Like kernel() but with NTFF profiling; returns (out, exec_time_ns)."""
    in_maps = _prepare_in_maps(**inputs)
    res = _run(in_maps, trace=True)
    return _assemble(res.results), res.exec_time_ns
